# revision 15
# baseline (speedup 1.0000x reference)
"""Trainium2 Bass kernel for nn_DefuzzyLayer2 (dense_mlp).

Computes out[b,o] = sum_d x[b,d]^2 * W2[d,o] + sum_d x[b,d] * W1[d,o]
                    + sum_d bias[d,o]
for x [8192, 512], W1/W2/bias [512, 512], all float32.

Sharding: data-parallel over batch across 8 NeuronCores (1024 rows each);
the three (512,512) parameter matrices are replicated.

DMA layout: HBM descriptors are fastest with long contiguous runs, so every
load/store is row-LINEAR: partition p holds consecutive DRAM rows (8KB runs)
instead of the classic strided [p, ko, n] layout (2KB runs). Consequences:
  - weights: w_sb[p, r*512+n] = W[4p+r, n]; contraction chunk r covers
    d = 4p + r (d = r mod 4), a perfectly valid k-tile.
  - x quarters: xq[p, r*512+d] = x[256q + 2p + r, d]; batch rows are
    permuted within the quarter and un-permuted for free via the matching
    linear store of the output staging tile.
  - the PE transposes pick strided columns d = 4j + r of each x slice so
    the xT chunks align with the weight chunks.

Per 128-row slice: 4 PE transposes (strided cols) -> PSUM; copy + square
to SBUF (float32r rounding); 8 accumulating float32r matmuls (1 cycle/row
PE fast path); DVE adds the precomputed bias row into the staging tile.
Bias row: 4 matmuls against an all-ones [128,128] stationary operand
(reduce over partitions + broadcast to 128 partitions in one shot).
"""

import os

import numpy as np

import concourse.mybir as mybir
import concourse.tile as tile
from concourse import bacc
from concourse.bass_utils import run_bass_kernel_spmd
from concourse.masks import make_identity

P = 128
B_TOTAL = 8192
D = 512
O = 512
N_CORES = 8
B_SHARD = B_TOTAL // N_CORES  # 1024
KO = D // P  # 4 contraction chunks
NQ = 4  # x quarters per core
RQ = B_SHARD // NQ // P  # 2 row-slices per quarter

F32 = mybir.dt.float32

# float32r streams 1 row/cycle through the PE at N>=256 (fp32 takes 4);
# set KERNEL_FP32=1 to fall back to exact fp32 matmuls.
USE_FP32R = os.environ.get("KERNEL_FP32", "0") != "1"
MM_DT = mybir.dt.float32r if USE_FP32R else F32


def build_bass():
    nc = bacc.Bacc("TRN2", target_bir_lowering=False, debug=False,
                   num_devices=N_CORES)

    x_d = nc.dram_tensor("x", [B_SHARD, D], F32, kind="ExternalInput").ap()
    w1_d = nc.dram_tensor("w1", [D, O], F32, kind="ExternalInput").ap()
    w2_d = nc.dram_tensor("w2", [D, O], F32, kind="ExternalInput").ap()
    b_d = nc.dram_tensor("bias", [D, O], F32, kind="ExternalInput").ap()
    out_d = nc.dram_tensor("out", [B_SHARD, O], F32, kind="ExternalOutput").ap()

    # Row-linear views: partition p <-> consecutive DRAM rows.
    xlin = x_d.rearrange("(q p r) d -> q p (r d)", q=NQ, p=P)      # 8KB runs
    olin = out_d.rearrange("(q p r) n -> q p (r n)", q=NQ, p=P)    # 8KB runs
    wlin = {
        "w1": w1_d.rearrange("(p r) n -> p (r n)", p=P),           # 8KB runs
        "w2": w2_d.rearrange("(p r) n -> p (r n)", p=P),
        "b": b_d.rearrange("(p r) n -> p (r n)", p=P),
    }

    with tile.TileContext(nc) as tc:
        with (
            tc.tile_pool(name="consts", bufs=1) as consts,
            tc.tile_pool(name="wload", bufs=1) as wload,
            tc.tile_pool(name="xin", bufs=NQ) as xin,
            tc.tile_pool(name="xt", bufs=RQ * NQ) as xtp,
            tc.tile_pool(name="ost", bufs=NQ) as ost,
            tc.tile_pool(name="pst", bufs=4, space="PSUM") as pst,
            tc.tile_pool(name="pso", bufs=3, space="PSUM") as pso,
            tc.tile_pool(name="psb", bufs=1, space="PSUM") as psb,
        ):
            ident = consts.tile([P, P], F32)
            make_identity(nc, ident[:])
            ones_stage = wload.tile([P, P], F32, tag="ones_stage")
            nc.vector.memset(ones_stage[:], 1.0)
            ones = consts.tile([P, P], MM_DT)
            nc.vector.tensor_copy(out=ones[:], in_=ones_stage[:])

            # x quarters: 4 linear 512KB loads on the Sync HWDGE queue.
            xqs = []
            for q in range(NQ):
                xq = xin.tile([P, RQ * D], F32, tag="xq")
                nc.sync.dma_start(xq[:], xlin[q])
                xqs.append(xq)

            # Weights/bias: linear 1MB loads; w1 on Sync, w2/bias on the
            # Activation HWDGE queue; rounding casts to float32r on DVE/ACT.
            w_sb = {}
            for name, issuer, caster in (
                ("w1", nc.sync, nc.vector.tensor_copy),
                ("w2", nc.scalar, nc.vector.tensor_copy),
                ("b", nc.scalar, nc.vector.tensor_copy),
            ):
                stage = wload.tile([P, KO * O], F32, tag=f"{name}_stage")
                issuer.dma_start(stage[:], wlin[name])
                wt = consts.tile([P, KO * O], MM_DT, tag=f"{name}_sb")
                caster(out=wt[:], in_=stage[:])
                w_sb[name] = wt

            # Transposes: slice (q, r) holds batch rows b = 256q + 2p + r.
            # Chunk rr takes strided columns d = 4j + rr so xT partitions
            # align with the linear weight chunks.
            slices = [(q, r) for q in range(NQ) for r in range(RQ)]
            xts, x2ts = {}, {}
            for q, r in slices:
                xs4 = xqs[q].rearrange("p (r dd four) -> p r dd four",
                                       r=RQ, four=KO)
                xt_ps = pst.tile([P, D], F32, tag="xt_ps")
                for rr in range(KO):
                    nc.tensor.transpose(xt_ps[:, rr * P:(rr + 1) * P],
                                        xs4[:, r, :, rr], ident[:])
                xt = xtp.tile([P, D], MM_DT, tag="xt")
                nc.vector.tensor_copy(out=xt[:], in_=xt_ps[:])
                x2t = xtp.tile([P, D], MM_DT, tag="x2t")
                nc.scalar.square(x2t[:], xt_ps[:])
                xts[(q, r)] = xt
                x2ts[(q, r)] = x2t

            # bias_bcast[m, n] = sum_d bias[d, n] for every m:
            # ones[128,128].T @ bias chunk, accumulated over the 4 chunks.
            bias_ps = psb.tile([P, O], F32)
            for r in range(KO):
                nc.tensor.matmul(bias_ps[:], lhsT=ones[:],
                                 rhs=w_sb["b"][:, r * O:(r + 1) * O],
                                 start=(r == 0), stop=(r == KO - 1))
            bias_sb = consts.tile([P, O], F32)
            nc.scalar.copy(bias_sb[:], bias_ps[:])

            # Main matmul stream + bias add into the linear staging tile;
            # one linear 512KB store per quarter (Sync queue).
            stages = []
            for q in range(NQ):
                ostage = ost.tile([P, RQ * O], F32, tag="ostage",
                                  name=f"ostage_{q}")
                stages.append(ostage)
            for q, r in slices:
                out_ps = pso.tile([P, O], F32, tag="out_ps")
                for rr in range(KO):
                    nc.tensor.matmul(out_ps[:],
                                     lhsT=xts[(q, r)][:, rr * P:(rr + 1) * P],
                                     rhs=w_sb["w1"][:, rr * O:(rr + 1) * O],
                                     start=(rr == 0), stop=False)
                for rr in range(KO):
                    nc.tensor.matmul(out_ps[:],
                                     lhsT=x2ts[(q, r)][:, rr * P:(rr + 1) * P],
                                     rhs=w_sb["w2"][:, rr * O:(rr + 1) * O],
                                     start=False, stop=(rr == KO - 1))
                nc.vector.tensor_add(out=stages[q][:, r * O:(r + 1) * O],
                                     in0=out_ps[:], in1=bias_sb[:])
                if r == RQ - 1:
                    nc.sync.dma_start(olin[q], stages[q][:])

    # Legalize sync (HW allows at most one wait per instruction), allocate
    # registers, etc.
    nc.compile()
    return nc


_NC_CACHE = None


def _get_nc():
    global _NC_CACHE
    if _NC_CACHE is None:
        _NC_CACHE = build_bass()
    return _NC_CACHE


def run(x, rules_outcome, bias, rules_outcome_2, **spmd_kwargs):
    """Run the kernel; returns (output, BassKernelResults)."""
    x = np.ascontiguousarray(x, dtype=np.float32)
    w1 = np.ascontiguousarray(rules_outcome, dtype=np.float32)
    w2 = np.ascontiguousarray(rules_outcome_2, dtype=np.float32)
    b = np.ascontiguousarray(bias, dtype=np.float32)

    nc = _get_nc()
    in_maps = [
        {
            "x": x[i * B_SHARD:(i + 1) * B_SHARD],
            "w1": w1,
            "w2": w2,
            "bias": b,
        }
        for i in range(N_CORES)
    ]
    res = run_bass_kernel_spmd(nc, in_maps, list(range(N_CORES)), **spmd_kwargs)
    out = np.concatenate([np.asarray(r["out"]) for r in res.results], axis=0)
    return out, res


def kernel(x, rules_outcome, bias, rules_outcome_2):
    try:
        out, _ = run(x, rules_outcome, bias, rules_outcome_2)
    except Exception:
        # Transient device errors (e.g. NRT_EXEC_UNIT_UNRECOVERABLE) have
        # been observed to succeed on retry.
        out, _ = run(x, rules_outcome, bias, rules_outcome_2)
    return out


# revision 17
# speedup vs baseline: 1.0776x; 1.0776x over previous
"""Trainium2 Bass kernel for nn_DefuzzyLayer2 (dense_mlp).

Computes out[b,o] = sum_d x[b,d]^2 * W2[d,o] + sum_d x[b,d] * W1[d,o]
                    + sum_d bias[d,o]
for x [8192, 512], W1/W2/bias [512, 512], all float32.

Sharding: data-parallel over batch across 8 NeuronCores (1024 rows each);
the three (512,512) parameter matrices are replicated.

DMA layout: HBM descriptors are fastest with long contiguous runs, so every
load/store is row-LINEAR: partition p holds consecutive DRAM rows (8KB runs)
instead of the classic strided [p, ko, n] layout (2KB runs). Consequences:
  - weights: w_sb[p, r*512+n] = W[4p+r, n]; contraction chunk r covers
    d = 4p + r (d = r mod 4), a perfectly valid k-tile.
  - x quarters: xq[p, r*512+d] = x[256q + 2p + r, d]; batch rows are
    permuted within the quarter and un-permuted for free via the matching
    linear store of the output staging tile.
  - the PE transposes pick strided columns d = 4j + r of each x slice so
    the xT chunks align with the weight chunks.

Per 128-row slice: 4 PE transposes (strided cols) -> PSUM; copy + square
to SBUF (float32r rounding); 8 accumulating float32r matmuls (1 cycle/row
PE fast path); DVE adds the precomputed bias row into the staging tile.
Bias row: 4 matmuls against an all-ones [128,128] stationary operand
(reduce over partitions + broadcast to 128 partitions in one shot).
"""

import os

import numpy as np

import concourse.mybir as mybir
import concourse.tile as tile
from concourse import bacc
from concourse.bass_utils import run_bass_kernel_spmd
from concourse.masks import make_identity

P = 128
B_TOTAL = 8192
D = 512
O = 512
N_CORES = 8
B_SHARD = B_TOTAL // N_CORES  # 1024
KO = D // P  # 4 contraction chunks
NQ = 4  # x quarters per core
RQ = B_SHARD // NQ // P  # 2 row-slices per quarter

F32 = mybir.dt.float32

# float32r streams 1 row/cycle through the PE at N>=256 (fp32 takes 4);
# set KERNEL_FP32=1 to fall back to exact fp32 matmuls.
USE_FP32R = os.environ.get("KERNEL_FP32", "0") != "1"
MM_DT = mybir.dt.float32r if USE_FP32R else F32


def build_bass():
    nc = bacc.Bacc("TRN2", target_bir_lowering=False, debug=False,
                   num_devices=N_CORES)

    x_d = nc.dram_tensor("x", [B_SHARD, D], F32, kind="ExternalInput").ap()
    w1_d = nc.dram_tensor("w1", [D, O], F32, kind="ExternalInput").ap()
    w2_d = nc.dram_tensor("w2", [D, O], F32, kind="ExternalInput").ap()
    b_d = nc.dram_tensor("bias", [D, O], F32, kind="ExternalInput").ap()
    out_d = nc.dram_tensor("out", [B_SHARD, O], F32, kind="ExternalOutput").ap()

    # Row-linear views: partition p <-> consecutive DRAM rows.
    xlin = x_d.rearrange("(q p r) d -> q p (r d)", q=NQ, p=P)      # 8KB runs
    olin = out_d.rearrange("(q p r) n -> q p (r n)", q=NQ, p=P)    # 8KB runs
    wlin = {
        "w1": w1_d.rearrange("(p r) n -> p (r n)", p=P),           # 8KB runs
        "w2": w2_d.rearrange("(p r) n -> p (r n)", p=P),
        "b": b_d.rearrange("(p r) n -> p (r n)", p=P),
    }

    with tile.TileContext(nc) as tc:
        with (
            tc.tile_pool(name="consts", bufs=1) as consts,
            tc.tile_pool(name="wload", bufs=1) as wload,
            tc.tile_pool(name="xin", bufs=NQ) as xin,
            tc.tile_pool(name="xt", bufs=RQ * NQ) as xtp,
            tc.tile_pool(name="ost", bufs=NQ) as ost,
            tc.tile_pool(name="pst", bufs=3, space="PSUM") as pst,
            tc.tile_pool(name="pso", bufs=3, space="PSUM") as pso,
            tc.tile_pool(name="psb", bufs=1, space="PSUM") as psb,
        ):
            ident = consts.tile([P, P], F32)
            make_identity(nc, ident[:])
            ones_stage = wload.tile([P, P], F32, tag="ones_stage")
            nc.vector.memset(ones_stage[:], 1.0)
            ones = consts.tile([P, P], MM_DT)
            nc.vector.tensor_copy(out=ones[:], in_=ones_stage[:])

            # x quarters: 4 linear 512KB loads, alternating between the
            # Sync and ACT HWDGE queues so consecutive quarters stream
            # concurrently (the PE otherwise stalls waiting for quarter 1).
            xqs = []
            for q in range(NQ):
                xq = xin.tile([P, RQ * D], F32, tag="xq")
                (nc.sync if q % 2 == 0 else nc.scalar).dma_start(xq[:], xlin[q])
                xqs.append(xq)

            # Weights/bias: linear 1MB loads; w1 on Sync, w2/bias on the
            # Activation HWDGE queue; rounding casts to float32r on DVE/ACT.
            w_sb = {}
            for name, issuer, caster in (
                ("w1", nc.sync, nc.vector.tensor_copy),
                ("w2", nc.scalar, nc.vector.tensor_copy),
                ("b", nc.scalar, nc.vector.tensor_copy),
            ):
                stage = wload.tile([P, KO * O], F32, tag=f"{name}_stage")
                issuer.dma_start(stage[:], wlin[name])
                wt = consts.tile([P, KO * O], MM_DT, tag=f"{name}_sb")
                caster(out=wt[:], in_=stage[:])
                w_sb[name] = wt

            # bias_bcast[m, n] = sum_d bias[d, n] for every m:
            # ones[128,128].T @ bias chunk, accumulated over the 4 chunks.
            bias_ps = psb.tile([P, O], F32)
            for r in range(KO):
                nc.tensor.matmul(bias_ps[:], lhsT=ones[:],
                                 rhs=w_sb["b"][:, r * O:(r + 1) * O],
                                 start=(r == 0), stop=(r == KO - 1))
            bias_sb = consts.tile([P, O], F32)
            nc.scalar.copy(bias_sb[:], bias_ps[:])

            # Transposes: slice (q, r) holds batch rows b = 256q + 2p + r.
            # Chunk rr takes strided columns d = 4j + rr so xT partitions
            # align with the linear weight chunks.
            slices = [(q, r) for q in range(NQ) for r in range(RQ)]
            xts, x2ts = {}, {}
            for q, r in slices:
                xs4 = xqs[q].rearrange("p (r dd four) -> p r dd four",
                                       r=RQ, four=KO)
                xt_ps = pst.tile([P, D], F32, tag="xt_ps")
                for rr in range(KO):
                    nc.tensor.transpose(xt_ps[:, rr * P:(rr + 1) * P],
                                        xs4[:, r, :, rr], ident[:])
                xt = xtp.tile([P, D], MM_DT, tag="xt")
                nc.vector.tensor_copy(out=xt[:], in_=xt_ps[:])
                x2t = xtp.tile([P, D], MM_DT, tag="x2t")
                nc.scalar.square(x2t[:], xt_ps[:])
                xts[(q, r)] = xt
                x2ts[(q, r)] = x2t

            # Main matmul stream + bias add into the linear staging tile;
            # one linear 512KB store per quarter (Sync queue).
            stages = []
            for q in range(NQ):
                ostage = ost.tile([P, RQ * O], F32, tag="ostage",
                                  name=f"ostage_{q}")
                stages.append(ostage)
            for q, r in slices:
                out_ps = pso.tile([P, O], F32, tag="out_ps")
                for rr in range(KO):
                    nc.tensor.matmul(out_ps[:],
                                     lhsT=xts[(q, r)][:, rr * P:(rr + 1) * P],
                                     rhs=w_sb["w1"][:, rr * O:(rr + 1) * O],
                                     start=(rr == 0), stop=False)
                for rr in range(KO):
                    nc.tensor.matmul(out_ps[:],
                                     lhsT=x2ts[(q, r)][:, rr * P:(rr + 1) * P],
                                     rhs=w_sb["w2"][:, rr * O:(rr + 1) * O],
                                     start=False, stop=(rr == KO - 1))
                nc.vector.tensor_add(out=stages[q][:, r * O:(r + 1) * O],
                                     in0=out_ps[:], in1=bias_sb[:])
                if q < NQ - 1:
                    if r == RQ - 1:
                        nc.sync.dma_start(olin[q], stages[q][:])
                else:
                    # last quarter: store each row-slice as soon as its bias
                    # add lands, so only ~256KB trails the final matmul
                    nc.sync.dma_start(olin[q][:, r * O:(r + 1) * O],
                                      stages[q][:, r * O:(r + 1) * O])

    # Legalize sync (HW allows at most one wait per instruction), allocate
    # registers, etc.
    nc.compile()
    return nc


_NC_CACHE = None


def _get_nc():
    global _NC_CACHE
    if _NC_CACHE is None:
        _NC_CACHE = build_bass()
    return _NC_CACHE


def run(x, rules_outcome, bias, rules_outcome_2, **spmd_kwargs):
    """Run the kernel; returns (output, BassKernelResults)."""
    x = np.ascontiguousarray(x, dtype=np.float32)
    w1 = np.ascontiguousarray(rules_outcome, dtype=np.float32)
    w2 = np.ascontiguousarray(rules_outcome_2, dtype=np.float32)
    b = np.ascontiguousarray(bias, dtype=np.float32)

    nc = _get_nc()
    in_maps = [
        {
            "x": x[i * B_SHARD:(i + 1) * B_SHARD],
            "w1": w1,
            "w2": w2,
            "bias": b,
        }
        for i in range(N_CORES)
    ]
    res = run_bass_kernel_spmd(nc, in_maps, list(range(N_CORES)), **spmd_kwargs)
    out = np.concatenate([np.asarray(r["out"]) for r in res.results], axis=0)
    return out, res


def kernel(x, rules_outcome, bias, rules_outcome_2):
    try:
        out, _ = run(x, rules_outcome, bias, rules_outcome_2)
    except Exception:
        # Transient device errors (e.g. NRT_EXEC_UNIT_UNRECOVERABLE) have
        # been observed to succeed on retry.
        out, _ = run(x, rules_outcome, bias, rules_outcome_2)
    return out
